# revision 17
# baseline (speedup 1.0000x reference)
"""GCNConv (N=20000, E=320000, D=1024) on 8 trn2 NeuronCores.

out = segment_sum(norm * h[col] -> row),  h = x @ W^T + b,
norm = deg^-1/2[row] * deg^-1/2[col], with self-loops added.

Sharding: nodes are assigned to the 160 (core, block) bins by a balanced
partition (LPT on in-degree totals, then a per-block-row vector rebalance
across cores that keeps chunk labels invariant), so the shared gather
schedule has near-zero padding; the host un-permutes the output.
Per core: h_scaled = (dis*x) @ W^T + dis (x) b  (bf16 matmul + exact fp32
rank-1 bias, bf16 store).  The AllGather of h_scaled runs in 2 chunks
interleaved with the h compute; edges are partitioned by destination
block and sub-partitioned by source chunk (chunk-major tile order).
Per-segment bulk dma_gather calls (2KB rows) cycle 4 SWDGE queues with a
6-deep buffer pipeline; segment-sum per (chunk, block) via 0/1 selection
matmuls (iota==dest_local) accumulating in fp32 PSUM, a bf16 SBUF
accumulator across chunks, and a dis[dest] scale on the Activation engine
on the way out (the last chunk's add is fused into the output in fp32).
"""

import numpy as np
import ml_dtypes

import concourse.bacc as bacc
import concourse.mybir as mybir
import concourse.tile as tile
from concourse import bass
from concourse import bass_utils

N = 20000
E = 320000
D = 1024
NC = 8
NPC = N // NC            # 2500 nominal nodes per core
NBLK = 20                # dest blocks of 128 per core
NPCP = NBLK * 128        # 2560 padded slots per core
P = 128
KT = D // P              # 8 contraction tiles
CHG = 2                  # source chunks (blocks 0-9 / 10-19)
GCALL = 8                # close a gather call above this many tiles
NQ = 4

_cache = {}


def _partition_nodes(row, col):
    """Assign nodes to (core, blk, pos) bins balancing per-(blk, chunk)
    in-degree across cores.  Returns (node_core, node_blk, node_pos)."""
    indeg = np.bincount(row, minlength=N).astype(np.int64)
    # phase A: LPT on total indeg into 160 bins (row-major: bin = blk*NC+c)
    nbins = NC * NBLK
    order = np.argsort(-indeg, kind="stable")
    import heapq

    heap = [(0, b) for b in range(nbins)]
    heapq.heapify(heap)
    counts = np.zeros(nbins, dtype=np.int64)
    node_bin = np.empty(N, dtype=np.int64)
    for n in order:
        while True:
            load, b = heapq.heappop(heap)
            if counts[b] < P:
                break
        node_bin[n] = b
        counts[b] += 1
        if counts[b] < P:
            heapq.heappush(heap, (load + indeg[n], b))
    blk_of = node_bin // NC  # 0..19  (fixes chunk labels)
    # phase B: per blk-row, re-balance its nodes across the 8 cores on the
    # (d0, d1) vector (source-chunk labels depend only on blk_of, invariant
    # under within-row reassignment)
    src_ch = (blk_of[col] >= NBLK // 2).astype(np.int64)  # chunk of each edge
    d = np.zeros((N, CHG), dtype=np.int64)
    np.add.at(d, (row, src_ch), 1)
    node_core = np.empty(N, dtype=np.int64)
    node_pos = np.empty(N, dtype=np.int64)
    for bk in range(NBLK):
        nodes = np.where(blk_of == bk)[0]
        nodes = nodes[np.argsort(-(d[nodes].sum(axis=1)), kind="stable")]
        loads = np.zeros((NC, CHG), dtype=np.int64)
        cnt = np.zeros(NC, dtype=np.int64)
        for n in nodes:
            best, bestcost = -1, None
            for c in range(NC):
                if cnt[c] >= P:
                    continue
                cost = np.max(loads[c] + d[n])
                if bestcost is None or cost < bestcost:
                    best, bestcost = c, cost
            node_core[n] = best
            node_pos[n] = cnt[best]
            loads[best] += d[n]
            cnt[best] += 1
    return node_core, blk_of, node_pos


def _preprocess(x, edge_index, W, b):
    x = np.asarray(x, dtype=np.float32)
    ei = np.asarray(edge_index)
    W = np.asarray(W, dtype=np.float32)
    b = np.asarray(b, dtype=np.float32)

    row = ei[0].astype(np.int64)
    col = ei[1].astype(np.int64)
    deg = (np.bincount(row, minlength=N) + 1).astype(np.float32)
    dis = deg ** -0.5

    node_core, node_blk, node_pos = _partition_nodes(row, col)
    lrow_of = node_blk * P + node_pos          # local padded row (0..2559)

    s_src = node_core[col]
    l_src = lrow_of[col]
    chk = (node_blk[col] >= NBLK // 2).astype(np.int64)
    RCH = NPCP // CHG
    wcr = (s_src * RCH + (l_src - chk * RCH)).astype(np.int32)

    core_d = node_core[row].astype(np.int32)
    rl = lrow_of[row].astype(np.int32)

    seg_cols = {}
    seg_dl = {}
    for c in range(NC):
        m = core_d == c
        rl_c = rl[m]
        wcr_c = wcr[m]
        chk_c = chk[m]
        blk = rl_c // P
        key = chk_c * NBLK + blk
        order = np.argsort(key, kind="stable")
        rl_c, wcr_c, key = rl_c[order], wcr_c[order], key[order]
        bounds = np.searchsorted(key, np.arange(CHG * NBLK + 1))
        for ch in range(CHG):
            for bk in range(NBLK):
                s, e = bounds[ch * NBLK + bk], bounds[ch * NBLK + bk + 1]
                o2 = np.argsort(wcr_c[s:e], kind="stable")
                seg_cols[(c, ch, bk)] = wcr_c[s:e][o2]
                seg_dl[(c, ch, bk)] = (rl_c[s:e] - bk * P)[o2]

    T_mat = []
    for ch in range(CHG):
        rowT = []
        for bk in range(NBLK):
            mx = max(len(seg_cols[(c, ch, bk)]) for c in range(NC))
            rowT.append(-(-mx // P))
        T_mat.append(tuple(rowT))
    T_mat = tuple(T_mat)
    NT = sum(sum(r) for r in T_mat)

    dl = np.full((NC, NT, P), -1.0, dtype=np.float32)
    idx16 = np.zeros((NC, 16, NT * 8), dtype=np.int16)
    t0 = 0
    for ch in range(CHG):
        for bk in range(NBLK):
            Tb = T_mat[ch][bk]
            if Tb == 0:
                continue
            for c in range(NC):
                cc = seg_cols[(c, ch, bk)]
                dd = seg_dl[(c, ch, bk)]
                n = len(cc)
                flat_d = dl[c, t0 : t0 + Tb].reshape(-1)
                flat_d[:n] = dd.astype(np.float32)
                buf = np.zeros(Tb * P, dtype=np.int16)
                buf[:n] = cc.astype(np.int16)
                idx16[c, :, t0 * 8 : (t0 + Tb) * 8] = buf.reshape(Tb * 8, 16).T
            t0 += Tb

    WTb = np.zeros((D + P, D), dtype=ml_dtypes.bfloat16)
    WTb[:D] = W.T.astype(ml_dtypes.bfloat16)
    WTb[D:] = (b / P).astype(ml_dtypes.bfloat16)[None, :]

    # inverse permutation for output assembly
    perm = np.empty((NC, NPCP), dtype=np.int64)  # (core, lrow) -> node (or -1)
    perm.fill(-1)
    nodes = np.arange(N)
    perm[node_core, lrow_of] = nodes

    in_maps = []
    for c in range(NC):
        sel = perm[c]
        valid = sel >= 0
        idx = np.where(valid, sel, 0)
        dis_c = np.where(valid, dis[idx], 0.0).astype(np.float32)
        xs = x[idx] * dis_c[:, None]
        xs[~valid] = 0.0
        xT = np.zeros((D + P, NPCP), dtype=ml_dtypes.bfloat16)
        xT[:D] = xs.T.astype(ml_dtypes.bfloat16)
        xT[D:] = dis_c.astype(ml_dtypes.bfloat16)[None, :]
        disT = np.ascontiguousarray(
            dis_c.reshape(NBLK, P).T.astype(np.float32)
        )  # [128, NBLK]
        in_maps.append(
            {
                "xT": xT,
                "WT": WTb,
                "disT": disT,
                "idx16": np.ascontiguousarray(np.tile(idx16[c], (8, 1))),
                "dl": np.ascontiguousarray(dl[c].T),
            }
        )
    return T_mat, NT, in_maps, perm


def _build(T_mat, NT):
    f32 = mybir.dt.float32
    bf16 = mybir.dt.bfloat16
    i16 = mybir.dt.int16
    i32 = mybir.dt.int32
    JCH = NBLK // CHG

    tstart = {}
    t0 = 0
    for ch in range(CHG):
        for bk in range(NBLK):
            tstart[(ch, bk)] = t0
            t0 += T_mat[ch][bk]
    last_ch = {}
    for bk in range(NBLK):
        nz = [ch for ch in range(CHG) if T_mat[ch][bk] > 0]
        last_ch[bk] = nz[-1] if nz else -1

    calls = []
    for ch in range(CHG):
        cur, cur_tiles, cur_t0 = [], 0, None
        for bk in range(NBLK):
            Tb = T_mat[ch][bk]
            if Tb == 0:
                continue
            if cur and cur_tiles + Tb > GCALL:
                calls.append((ch, cur_t0, cur_tiles, cur))
                cur, cur_tiles, cur_t0 = [], 0, None
            if cur_t0 is None:
                cur_t0 = tstart[(ch, bk)]
            cur.append((bk, Tb, cur_tiles))
            cur_tiles += Tb
        if cur:
            calls.append((ch, cur_t0, cur_tiles, cur))
    GMAX = max(c[2] for c in calls)

    nc = bacc.Bacc("TRN2", target_bir_lowering=False, debug=False, num_devices=NC, num_swdge_queues=NQ)
    xT = nc.dram_tensor("xT", [D + P, NPCP], bf16, kind="ExternalInput").ap()
    WT = nc.dram_tensor("WT", [D + P, D], bf16, kind="ExternalInput").ap()
    disT = nc.dram_tensor("disT", [P, NBLK], f32, kind="ExternalInput").ap()
    idx16 = nc.dram_tensor("idx16", [P, NT * 8], i16, kind="ExternalInput").ap()
    dl = nc.dram_tensor("dl", [P, NT], f32, kind="ExternalInput").ap()
    yout = nc.dram_tensor("yout", [NPCP, D], f32, kind="ExternalOutput").ap()

    with tile.TileContext(nc) as tc:
        with tc.tile_pool(name="dram", bufs=1, space="DRAM") as dram, \
             tc.tile_pool(name="const", bufs=1) as const:
            RCH = NPCP // CHG
            h_ch = [dram.tile([RCH, D], bf16, name=f"h_ch{c_}") for c_ in range(CHG)]
            hg_ch = [
                dram.tile([NC * RCH, D], bf16, addr_space="Shared", name=f"hg_ch{c_}")
                for c_ in range(CHG)
            ]

            disT_sb = const.tile([P, NBLK], f32, name="disT_sb")
            nc.sync.dma_start(disT_sb[:], disT[:])
            ix_sb = const.tile([P, NT * 8], i16, name="ix_sb")
            nc.sync.dma_start(ix_sb[:], idx16[:])
            dl_sb = const.tile([P, NT], f32, name="dl_sb")
            nc.sync.dma_start(dl_sb[:], dl[:])
            TMAX = max(max(r) for r in T_mat)
            iota_rep = const.tile([P, TMAX * P], f32, name="iota_rep")
            with tc.tile_pool(name="tmpi", bufs=1) as tmpp:
                iota_i = tmpp.tile([P, TMAX * P], i32, name="iota_i")
                nc.gpsimd.iota(
                    iota_i[:], pattern=[[0, TMAX], [1, P]], channel_multiplier=0
                )
                nc.vector.tensor_copy(iota_rep[:], iota_i[:])

            acc_cm = tc.tile_pool(name="acc", bufs=1)
            accp = acc_cm.__enter__()
            acc = accp.tile([P, NBLK, D], bf16, name="acc")

            # ---------------- h phase (+ chunked AllGather) ----------------
            with tc.tile_pool(name="wt", bufs=1) as wtp, \
                 tc.tile_pool(name="xk", bufs=1) as xkp, \
                 tc.tile_pool(name="hps", bufs=2, space="PSUM") as hps, \
                 tc.tile_pool(name="hout", bufs=3) as houtp:
                wt_sb = wtp.tile([P, (KT + 1) * D], bf16, name="wt_sb")
                for k in range(KT + 1):
                    nc.sync.dma_start(
                        wt_sb[:, k * D : (k + 1) * D], WT[k * P : (k + 1) * P, :]
                    )
                xk_sb = xkp.tile([P, (KT + 1) * NPCP], bf16, name="xk_sb")
                for k in range(KT + 1):
                    nc.sync.dma_start(
                        xk_sb[:, k * NPCP : (k + 1) * NPCP],
                        xT[k * P : (k + 1) * P, :],
                    )
                chunks = [slice(s, min(s + 512, D)) for s in range(0, D, 512)]
                for j in range(NBLK):
                    ps = hps.tile([P, D], f32)
                    for k in range(KT + 1):
                        lhsT = xk_sb[:, k * NPCP + j * P : k * NPCP + (j + 1) * P]
                        for cs in chunks:
                            nc.tensor.matmul(
                                ps[:, cs],
                                lhsT=lhsT,
                                rhs=wt_sb[:, k * D + cs.start : k * D + cs.stop],
                                start=(k == 0),
                                stop=(k == KT),
                            )
                    hsb = houtp.tile([P, D], bf16)
                    nc.scalar.copy(hsb[:], ps[:])
                    nc.vector.tensor_copy(acc[:, j, :], ps[:])
                    ch = j // JCH
                    jo = j - ch * JCH
                    nc.sync.dma_start(h_ch[ch][jo * P : (jo + 1) * P, :], hsb[:])
                    if jo == JCH - 1:
                        nc.gpsimd.collective_compute(
                            "AllGather",
                            mybir.AluOpType.bypass,
                            replica_groups=[list(range(NC))],
                            ins=[h_ch[ch][:]],
                            outs=[hg_ch[ch][:]],
                        )

            # ---------------- aggregation phase ----------------
            with tc.tile_pool(name="gath", bufs=7) as gp, \
                 tc.tile_pool(name="sel", bufs=4) as selp, \
                 tc.tile_pool(name="aps", bufs=4, space="PSUM") as aps, \
                 tc.tile_pool(name="aout", bufs=3) as aoutp:
                gq = 0
                for (ch, ct0, ctiles, segs) in calls:
                    g = gp.tile([P, GMAX, D], bf16, tag="g")
                    nc.gpsimd.dma_gather(
                        g[:, 0:ctiles, :],
                        hg_ch[ch][:],
                        ix_sb[:, ct0 * 8 : (ct0 + ctiles) * 8],
                        ctiles * P,
                        ctiles * P,
                        D,
                        queue_num=gq,
                        single_packet=False,
                    )
                    gq = (gq + 1) % NQ
                    for (bk, Tb, off) in segs:
                        t0 = tstart[(ch, bk)]
                        selb = selp.tile([P, TMAX, P], bf16, tag="selb")
                        dlb = (
                            dl_sb[:, t0 : t0 + Tb]
                            .rearrange("p (t o) -> p t o", o=1)
                            .to_broadcast([P, Tb, P])
                        )
                        nc.vector.tensor_tensor(
                            out=selb[:, 0:Tb, :],
                            in0=iota_rep[:, : Tb * P].rearrange(
                                "p (t o) -> p t o", o=P
                            ),
                            in1=dlb,
                            op=mybir.AluOpType.is_equal,
                        )
                        ps = aps.tile([P, D], f32)
                        for i in range(Tb):
                            for cs in [
                                slice(s, min(s + 512, D)) for s in range(0, D, 512)
                            ]:
                                nc.tensor.matmul(
                                    ps[:, cs],
                                    lhsT=selb[:, i, :],
                                    rhs=g[:, off + i, cs],
                                    start=(i == 0),
                                    stop=(i == Tb - 1),
                                )
                        if ch == last_ch[bk]:
                            ob = aoutp.tile([P, D], f32)
                            nc.vector.tensor_add(
                                out=ob[:], in0=ps[:], in1=acc[:, bk, :]
                            )
                            nc.scalar.mul(ob[:], ob[:], disT_sb[:, bk : bk + 1])
                            nc.sync.dma_start(yout[bk * P : (bk + 1) * P, :], ob[:])
                        else:
                            nc.vector.tensor_add(
                                out=acc[:, bk, :], in0=ps[:], in1=acc[:, bk, :]
                            )
                for bk in range(NBLK):
                    if last_ch[bk] == -1:
                        ob = aoutp.tile([P, D], f32, tag="ob")
                        nc.scalar.mul(ob[:], acc[:, bk, :], disT_sb[:, bk : bk + 1])
                        nc.sync.dma_start(yout[bk * P : (bk + 1) * P, :], ob[:])
            acc_cm.__exit__(None, None, None)

    nc.compile()
    return nc


def kernel(x, edge_index, W, b):
    T_mat, NT, in_maps, perm = _preprocess(x, edge_index, W, b)
    key = (T_mat, NT)
    if key not in _cache:
        _cache[key] = _build(T_mat, NT)
    nc = _cache[key]
    res = bass_utils.run_bass_kernel_spmd(nc, in_maps, core_ids=list(range(NC)))
    out = np.empty((N, D), dtype=np.float32)
    for c in range(NC):
        sel = perm[c]
        valid = sel >= 0
        out[sel[valid]] = res.results[c]["yout"][valid]
    return out


# revision 18
# speedup vs baseline: 1.0102x; 1.0102x over previous
"""GCNConv (N=20000, E=320000, D=1024) on 8 trn2 NeuronCores.

out = segment_sum(norm * h[col] -> row),  h = x @ W^T + b,
norm = deg^-1/2[row] * deg^-1/2[col], with self-loops added.

Sharding: nodes are assigned to the 160 (core, block) bins by a balanced
partition (LPT on in-degree totals, then a per-block-row vector rebalance
across cores that keeps chunk labels invariant), so the shared gather
schedule has near-zero padding; the host un-permutes the output.
Per core: h_scaled = (dis*x) @ W^T + dis (x) b  (bf16 matmul + exact fp32
rank-1 bias, bf16 store).  The AllGather of h_scaled runs in 2 chunks
interleaved with the h compute; edges are partitioned by destination
block and sub-partitioned by source chunk (chunk-major tile order).
Per-segment bulk dma_gather calls (2KB rows) cycle 4 SWDGE queues with a
6-deep buffer pipeline; segment-sum per (chunk, block) via 0/1 selection
matmuls (iota==dest_local) accumulating in fp32 PSUM, a bf16 SBUF
accumulator across chunks, and a dis[dest] scale on the Activation engine
on the way out (the last chunk's add is fused into the output in fp32).
"""

import numpy as np
import ml_dtypes

import concourse.bacc as bacc
import concourse.mybir as mybir
import concourse.tile as tile
from concourse import bass
from concourse import bass_utils

N = 20000
E = 320000
D = 1024
NC = 8
NPC = N // NC            # 2500 nominal nodes per core
NBLK = 20                # dest blocks of 128 per core
NPCP = NBLK * 128        # 2560 padded slots per core
P = 128
KT = D // P              # 8 contraction tiles
CHG = 2                  # source chunks (blocks 0-9 / 10-19)
GCALL = 8                # close a gather call above this many tiles
NQ = 4

_cache = {}


def _partition_nodes(row, col):
    """Assign nodes to (core, blk, pos) bins balancing per-(blk, chunk)
    in-degree across cores.  Returns (node_core, node_blk, node_pos)."""
    indeg = np.bincount(row, minlength=N).astype(np.int64)
    # phase A: LPT on total indeg into 160 bins (row-major: bin = blk*NC+c)
    nbins = NC * NBLK
    order = np.argsort(-indeg, kind="stable")
    import heapq

    heap = [(0, b) for b in range(nbins)]
    heapq.heapify(heap)
    counts = np.zeros(nbins, dtype=np.int64)
    node_bin = np.empty(N, dtype=np.int64)
    for n in order:
        while True:
            load, b = heapq.heappop(heap)
            if counts[b] < P:
                break
        node_bin[n] = b
        counts[b] += 1
        if counts[b] < P:
            heapq.heappush(heap, (load + indeg[n], b))
    blk_of = node_bin // NC  # 0..19  (fixes chunk labels)
    # phase B: per blk-row, re-balance its nodes across the 8 cores on the
    # (d0, d1) vector (source-chunk labels depend only on blk_of, invariant
    # under within-row reassignment)
    src_ch = (blk_of[col] >= NBLK // 2).astype(np.int64)  # chunk of each edge
    d = np.zeros((N, CHG), dtype=np.int64)
    np.add.at(d, (row, src_ch), 1)
    node_core = np.empty(N, dtype=np.int64)
    node_pos = np.empty(N, dtype=np.int64)
    for bk in range(NBLK):
        nodes = np.where(blk_of == bk)[0]
        nodes = nodes[np.argsort(-(d[nodes].sum(axis=1)), kind="stable")]
        loads = np.zeros((NC, CHG), dtype=np.int64)
        cnt = np.zeros(NC, dtype=np.int64)
        for n in nodes:
            best, bestcost = -1, None
            for c in range(NC):
                if cnt[c] >= P:
                    continue
                cost = np.max(loads[c] + d[n])
                if bestcost is None or cost < bestcost:
                    best, bestcost = c, cost
            node_core[n] = best
            node_pos[n] = cnt[best]
            loads[best] += d[n]
            cnt[best] += 1
    return node_core, blk_of, node_pos


def _preprocess(x, edge_index, W, b):
    x = np.asarray(x, dtype=np.float32)
    ei = np.asarray(edge_index)
    W = np.asarray(W, dtype=np.float32)
    b = np.asarray(b, dtype=np.float32)

    row = ei[0].astype(np.int64)
    col = ei[1].astype(np.int64)
    deg = (np.bincount(row, minlength=N) + 1).astype(np.float32)
    dis = deg ** -0.5

    node_core, node_blk, node_pos = _partition_nodes(row, col)
    lrow_of = node_blk * P + node_pos          # local padded row (0..2559)

    s_src = node_core[col]
    l_src = lrow_of[col]
    chk = (node_blk[col] >= NBLK // 2).astype(np.int64)
    RCH = NPCP // CHG
    wcr = (s_src * RCH + (l_src - chk * RCH)).astype(np.int32)

    core_d = node_core[row].astype(np.int32)
    rl = lrow_of[row].astype(np.int32)

    seg_cols = {}
    seg_dl = {}
    for c in range(NC):
        m = core_d == c
        rl_c = rl[m]
        wcr_c = wcr[m]
        chk_c = chk[m]
        blk = rl_c // P
        key = chk_c * NBLK + blk
        order = np.argsort(key, kind="stable")
        rl_c, wcr_c, key = rl_c[order], wcr_c[order], key[order]
        bounds = np.searchsorted(key, np.arange(CHG * NBLK + 1))
        for ch in range(CHG):
            for bk in range(NBLK):
                s, e = bounds[ch * NBLK + bk], bounds[ch * NBLK + bk + 1]
                o2 = np.argsort(wcr_c[s:e], kind="stable")
                seg_cols[(c, ch, bk)] = wcr_c[s:e][o2]
                seg_dl[(c, ch, bk)] = (rl_c[s:e] - bk * P)[o2]

    T_mat = []
    for ch in range(CHG):
        rowT = []
        for bk in range(NBLK):
            mx = max(len(seg_cols[(c, ch, bk)]) for c in range(NC))
            rowT.append(-(-mx // P))
        T_mat.append(tuple(rowT))
    T_mat = tuple(T_mat)
    NT = sum(sum(r) for r in T_mat)

    dl = np.full((NC, NT, P), -1.0, dtype=np.float32)
    idx16 = np.zeros((NC, 16, NT * 8), dtype=np.int16)
    t0 = 0
    for ch in range(CHG):
        for bk in range(NBLK):
            Tb = T_mat[ch][bk]
            if Tb == 0:
                continue
            for c in range(NC):
                cc = seg_cols[(c, ch, bk)]
                dd = seg_dl[(c, ch, bk)]
                n = len(cc)
                flat_d = dl[c, t0 : t0 + Tb].reshape(-1)
                flat_d[:n] = dd.astype(np.float32)
                buf = np.zeros(Tb * P, dtype=np.int16)
                buf[:n] = cc.astype(np.int16)
                idx16[c, :, t0 * 8 : (t0 + Tb) * 8] = buf.reshape(Tb * 8, 16).T
            t0 += Tb

    WTb = np.zeros((D + P, D), dtype=ml_dtypes.bfloat16)
    WTb[:D] = W.T.astype(ml_dtypes.bfloat16)
    WTb[D:] = (b / P).astype(ml_dtypes.bfloat16)[None, :]

    # inverse permutation for output assembly
    perm = np.empty((NC, NPCP), dtype=np.int64)  # (core, lrow) -> node (or -1)
    perm.fill(-1)
    nodes = np.arange(N)
    perm[node_core, lrow_of] = nodes

    in_maps = []
    for c in range(NC):
        sel = perm[c]
        valid = sel >= 0
        idx = np.where(valid, sel, 0)
        dis_c = np.where(valid, dis[idx], 0.0).astype(np.float32)
        xs = x[idx] * dis_c[:, None]
        xs[~valid] = 0.0
        xT = np.zeros((D + P, NPCP), dtype=ml_dtypes.bfloat16)
        xT[:D] = xs.T.astype(ml_dtypes.bfloat16)
        xT[D:] = dis_c.astype(ml_dtypes.bfloat16)[None, :]
        disT = np.ascontiguousarray(
            dis_c.reshape(NBLK, P).T.astype(np.float32)
        )  # [128, NBLK]
        in_maps.append(
            {
                "xT": xT,
                "WT": WTb,
                "disT": disT,
                "idx16": np.ascontiguousarray(np.tile(idx16[c], (8, 1))),
                "dl": np.ascontiguousarray(dl[c].T),
            }
        )
    return T_mat, NT, in_maps, perm


def _build(T_mat, NT):
    f32 = mybir.dt.float32
    bf16 = mybir.dt.bfloat16
    i16 = mybir.dt.int16
    i32 = mybir.dt.int32
    JCH = NBLK // CHG

    tstart = {}
    t0 = 0
    for ch in range(CHG):
        for bk in range(NBLK):
            tstart[(ch, bk)] = t0
            t0 += T_mat[ch][bk]
    last_ch = {}
    for bk in range(NBLK):
        nz = [ch for ch in range(CHG) if T_mat[ch][bk] > 0]
        last_ch[bk] = nz[-1] if nz else -1

    calls = []
    for ch in range(CHG):
        cur, cur_tiles, cur_t0 = [], 0, None
        for bk in range(NBLK):
            Tb = T_mat[ch][bk]
            if Tb == 0:
                continue
            if cur and cur_tiles + Tb > GCALL:
                calls.append((ch, cur_t0, cur_tiles, cur))
                cur, cur_tiles, cur_t0 = [], 0, None
            if cur_t0 is None:
                cur_t0 = tstart[(ch, bk)]
            cur.append((bk, Tb, cur_tiles))
            cur_tiles += Tb
        if cur:
            calls.append((ch, cur_t0, cur_tiles, cur))
    GMAX = max(c[2] for c in calls)

    nc = bacc.Bacc("TRN2", target_bir_lowering=False, debug=False, num_devices=NC, num_swdge_queues=NQ)
    xT = nc.dram_tensor("xT", [D + P, NPCP], bf16, kind="ExternalInput").ap()
    WT = nc.dram_tensor("WT", [D + P, D], bf16, kind="ExternalInput").ap()
    disT = nc.dram_tensor("disT", [P, NBLK], f32, kind="ExternalInput").ap()
    idx16 = nc.dram_tensor("idx16", [P, NT * 8], i16, kind="ExternalInput").ap()
    dl = nc.dram_tensor("dl", [P, NT], f32, kind="ExternalInput").ap()
    yout = nc.dram_tensor("yout", [NPCP, D], f32, kind="ExternalOutput").ap()

    with tile.TileContext(nc) as tc:
        with tc.tile_pool(name="dram", bufs=1, space="DRAM") as dram, \
             tc.tile_pool(name="const", bufs=1) as const:
            RCH = NPCP // CHG
            h_ch = [dram.tile([RCH, D], bf16, name=f"h_ch{c_}") for c_ in range(CHG)]
            hg_ch = [
                dram.tile([NC * RCH, D], bf16, addr_space="Shared", name=f"hg_ch{c_}")
                for c_ in range(CHG)
            ]

            disT_sb = const.tile([P, NBLK], f32, name="disT_sb")
            nc.sync.dma_start(disT_sb[:], disT[:])
            ix_sb = const.tile([P, NT * 8], i16, name="ix_sb")
            nc.sync.dma_start(ix_sb[:], idx16[:])
            dl_sb = const.tile([P, NT], f32, name="dl_sb")
            nc.sync.dma_start(dl_sb[:], dl[:])
            TMAX = max(max(r) for r in T_mat)
            iota_rep = const.tile([P, TMAX * P], f32, name="iota_rep")
            with tc.tile_pool(name="tmpi", bufs=1) as tmpp:
                iota_i = tmpp.tile([P, TMAX * P], i32, name="iota_i")
                nc.gpsimd.iota(
                    iota_i[:], pattern=[[0, TMAX], [1, P]], channel_multiplier=0
                )
                nc.vector.tensor_copy(iota_rep[:], iota_i[:])

            acc_cm = tc.tile_pool(name="acc", bufs=1)
            accp = acc_cm.__enter__()
            acc = accp.tile([P, NBLK, D], bf16, name="acc")

            # ---------------- h phase (+ chunked AllGather) ----------------
            with tc.tile_pool(name="wt", bufs=1) as wtp, \
                 tc.tile_pool(name="xk", bufs=1) as xkp, \
                 tc.tile_pool(name="hps", bufs=2, space="PSUM") as hps, \
                 tc.tile_pool(name="hout", bufs=3) as houtp:
                wt_sb = wtp.tile([P, (KT + 1) * D], bf16, name="wt_sb")
                for k in range(KT + 1):
                    nc.sync.dma_start(
                        wt_sb[:, k * D : (k + 1) * D], WT[k * P : (k + 1) * P, :]
                    )
                xk_sb = xkp.tile([P, (KT + 1) * NPCP], bf16, name="xk_sb")
                for k in range(KT + 1):
                    nc.sync.dma_start(
                        xk_sb[:, k * NPCP : (k + 1) * NPCP],
                        xT[k * P : (k + 1) * P, :],
                    )
                chunks = [slice(s, min(s + 512, D)) for s in range(0, D, 512)]
                for j in range(NBLK):
                    ps = hps.tile([P, D], f32)
                    for k in range(KT + 1):
                        lhsT = xk_sb[:, k * NPCP + j * P : k * NPCP + (j + 1) * P]
                        for cs in chunks:
                            nc.tensor.matmul(
                                ps[:, cs],
                                lhsT=lhsT,
                                rhs=wt_sb[:, k * D + cs.start : k * D + cs.stop],
                                start=(k == 0),
                                stop=(k == KT),
                            )
                    hsb = houtp.tile([P, D], bf16)
                    nc.scalar.copy(hsb[:], ps[:])
                    nc.vector.tensor_copy(acc[:, j, :], ps[:])
                    ch = j // JCH
                    jo = j - ch * JCH
                    nc.sync.dma_start(h_ch[ch][jo * P : (jo + 1) * P, :], hsb[:])
                    if jo == JCH - 1:
                        nc.gpsimd.collective_compute(
                            "AllGather",
                            mybir.AluOpType.bypass,
                            replica_groups=[list(range(NC))],
                            ins=[h_ch[ch][:]],
                            outs=[hg_ch[ch][:]],
                        )

            # ---------------- aggregation phase ----------------
            with tc.tile_pool(name="gath", bufs=6) as gp, \
                 tc.tile_pool(name="sel", bufs=3) as selp, \
                 tc.tile_pool(name="aps", bufs=3, space="PSUM") as aps, \
                 tc.tile_pool(name="aout", bufs=3) as aoutp:
                gq = 0
                for (ch, ct0, ctiles, segs) in calls:
                    g = gp.tile([P, GMAX, D], bf16, tag="g")
                    nc.gpsimd.dma_gather(
                        g[:, 0:ctiles, :],
                        hg_ch[ch][:],
                        ix_sb[:, ct0 * 8 : (ct0 + ctiles) * 8],
                        ctiles * P,
                        ctiles * P,
                        D,
                        queue_num=gq,
                        single_packet=False,
                    )
                    gq = (gq + 1) % NQ
                    for (bk, Tb, off) in segs:
                        t0 = tstart[(ch, bk)]
                        selb = selp.tile([P, TMAX, P], bf16, tag="selb")
                        dlb = (
                            dl_sb[:, t0 : t0 + Tb]
                            .rearrange("p (t o) -> p t o", o=1)
                            .to_broadcast([P, Tb, P])
                        )
                        nc.vector.tensor_tensor(
                            out=selb[:, 0:Tb, :],
                            in0=iota_rep[:, : Tb * P].rearrange(
                                "p (t o) -> p t o", o=P
                            ),
                            in1=dlb,
                            op=mybir.AluOpType.is_equal,
                        )
                        ps = aps.tile([P, D], f32)
                        for i in range(Tb):
                            for cs in [
                                slice(s, min(s + 512, D)) for s in range(0, D, 512)
                            ]:
                                nc.tensor.matmul(
                                    ps[:, cs],
                                    lhsT=selb[:, i, :],
                                    rhs=g[:, off + i, cs],
                                    start=(i == 0),
                                    stop=(i == Tb - 1),
                                )
                        if ch == last_ch[bk]:
                            ob = aoutp.tile([P, D], f32)
                            nc.vector.tensor_add(
                                out=ob[:], in0=ps[:], in1=acc[:, bk, :]
                            )
                            nc.scalar.mul(ob[:], ob[:], disT_sb[:, bk : bk + 1])
                            nc.sync.dma_start(yout[bk * P : (bk + 1) * P, :], ob[:])
                        else:
                            nc.vector.tensor_add(
                                out=acc[:, bk, :], in0=ps[:], in1=acc[:, bk, :]
                            )
                for bk in range(NBLK):
                    if last_ch[bk] == -1:
                        ob = aoutp.tile([P, D], f32, tag="ob")
                        nc.scalar.mul(ob[:], acc[:, bk, :], disT_sb[:, bk : bk + 1])
                        nc.sync.dma_start(yout[bk * P : (bk + 1) * P, :], ob[:])
            acc_cm.__exit__(None, None, None)

    nc.compile()
    return nc


def kernel(x, edge_index, W, b):
    T_mat, NT, in_maps, perm = _preprocess(x, edge_index, W, b)
    key = (T_mat, NT)
    if key not in _cache:
        _cache[key] = _build(T_mat, NT)
    nc = _cache[key]
    res = bass_utils.run_bass_kernel_spmd(nc, in_maps, core_ids=list(range(NC)))
    out = np.empty((N, D), dtype=np.float32)
    for c in range(NC):
        sel = perm[c]
        valid = sel >= 0
        out[sel[valid]] = res.results[c]["yout"][valid]
    return out
